# revision 40
# baseline (speedup 1.0000x reference)
"""Trainium2 Bass kernel for nn_H_ATT (GatedTrans pair-attention block).

Math (per example):
  HE = tanh(hist@W_hy+b_hy) * lrelu(hist@W_hg+b_hg)      [R, H]
  QE = tanh(ques@W_qy+b_qy) * lrelu(ques@W_qg+b_qg)      [R, H]
  numT[h,q] = sum_k HE[h,k]*QE[q,k]*W_att[k]
  denT[h,q] = sqrt(sum_k HE[h,k]^2 * QE[q,k]^2)
  sT = numT / den                   (b_att cancels in softmax)
  attU^T = exp(sT + maskT)          (causal mask additive; unnormalized)
  feat = (attU @ hist) / rowsum(attU)    [R, 2H]

Sharding: pure data parallel, 8 examples per core on 8 NeuronCores.
Default mode quantizes the four big [IN,H] weight matrices and the
activations to fp8 e4m3 (weights pre-scaled by 1024 to clear the e4m3
subnormal range; the 1/1024 descale is folded into the tanh/relu
activations) and runs the embedding GEMMs in DoubleRow perf mode.
The score/attention path stays in bf16 with fp32 PSUM accumulation.
All host-side layouts are partition-major so every DMA is a straight
[128, contiguous-bytes] copy.
"""

import numpy as np
import ml_dtypes

import bass_rust
import concourse.bass as bass
import concourse.mybir as mybir
import concourse.tile as tile
from concourse.vector_clock import ScopedClock

# ---------------------------------------------------------------------------
# Workaround: this walrus build accepts only ONE semaphore wait on an SP
# Drain, but TileContext's tail drain carries one wait per live semaphore.
# Split them across a chain of drains.
# ---------------------------------------------------------------------------


def _patched_drain_and_barrier(self, tick_clock, wait_clock):
    nc = self.nc
    drain_inst = nc.sync.drain()
    wait_clock.add_sem_waits(
        drain_inst.ins, ScopedClock({None: tick_clock.global_clock})
    )
    waits = list(drain_inst.ins.sync_info.on_wait)
    if len(waits) > 1:
        drain_inst.ins.sync_info = bass_rust.SyncInfo(
            on_wait=waits[:1], on_update=list(drain_inst.ins.sync_info.on_update)
        )
        for i in range(1, len(waits)):
            extra = nc.sync.drain()
            extra.ins.sync_info = bass_rust.SyncInfo(
                on_wait=waits[i : i + 1], on_update=[]
            )
    nc.all_engine_barrier()
    assert self.sems is not None
    popped = nc._tile_sem_poison_stack.pop()
    assert popped is self._sem_poison
    nc.clear_and_free_semaphores(list(self.sems.allocated().values()))
    nc.all_engine_barrier()


tile.TileContext._drain_and_barrier = _patched_drain_and_barrier


def _split_multi_waits(nc):
    """This walrus build accepts at most one semaphore wait per instruction.
    Hoist extra waits onto standalone EventSemaphore instructions inserted
    just before the owning instruction in the same engine's stream."""
    uid = [0]
    for f in nc.m.functions:
        for bb in f.blocks:
            out = []
            for inst in bb.instructions:
                si = inst.sync_info
                if si is not None and len(si.on_wait) > 1:
                    waits = list(si.on_wait)
                    for w in waits[:-1]:
                        nop = mybir.InstEventSemaphore(
                            name=f"I-waitsplit-{uid[0]}", ins=[], outs=[]
                        )
                        uid[0] += 1
                        nop.engine = inst.engine
                        nop.sync_info = bass_rust.SyncInfo(
                            on_wait=[w], on_update=[]
                        )
                        out.append(nop)
                    inst.sync_info = bass_rust.SyncInfo(
                        on_wait=[waits[-1]], on_update=list(si.on_update)
                    )
                out.append(inst)
            bb.instructions[:] = out

# ---------------------------------------------------------------------------

B, R, H, IN = 64, 32, 1024, 2048
NCORES = 8
BL = B // NCORES  # examples per core
BR = BL * R  # 256 rows per core
KC = IN // 128  # 16 contraction chunks
MC = H // 128  # 8 h chunks
NEG = -1.0e30
WSCALE = 1024.0  # fp8 weight pre-scale
ASCALE = 64.0  # fp8 W_att pre-scale (keeps qew out of e4m3 subnormals)

F32 = mybir.dt.float32
BF16 = mybir.dt.bfloat16
FP8 = mybir.dt.float8e4


def build_program(mode="fp8", zero_bias=True):
    fp8 = mode == "fp8"
    xdt = FP8 if fp8 else BF16
    s = (1.0 / WSCALE) if fp8 else 1.0
    EDT = FP8 if fp8 else BF16  # embedding storage for the num/den matmuls
    TDT = BF16  # ty/tg/qe temporaries

    nc = bass.Bass()
    qt_d = nc.dram_tensor("qt", [128, KC, BR], xdt, kind="ExternalInput")
    ht_d = nc.dram_tensor("ht", [128, KC, BR], xdt, kind="ExternalInput")
    hn_d = nc.dram_tensor("hn", [128, 2, IN], BF16, kind="ExternalInput")
    wh_d = nc.dram_tensor("wh", [MC, 128, 2, KC, 128], xdt, kind="ExternalInput")
    wq_d = nc.dram_tensor("wq", [MC, 128, 2, KC, 128], xdt, kind="ExternalInput")
    # packed consts: bqy|bqg|bhy|bhg|watt
    consts_d = nc.dram_tensor("consts", [128, 5 * MC], F32, kind="ExternalInput")
    # bf16 consts: eye128 | transposed additive causal mask
    consts2_d = nc.dram_tensor("consts2", [128, 256], BF16, kind="ExternalInput")
    # per-group output: IN cols of unnormalized feat + attention row sums
    feat_d = nc.dram_tensor("feat", [2, 128, IN + 16], BF16, kind="ExternalOutput")

    ACT = mybir.ActivationFunctionType
    ALU = mybir.AluOpType

    with tile.TileContext(nc) as tc:
        with (
            tc.tile_pool(name="big", bufs=1) as big,
            tc.tile_pool(name="wts", bufs=5) as wts,
            tc.tile_pool(name="tmp", bufs=3) as tmp,
            tc.tile_pool(name="sm", bufs=1) as sm,
        ):
            # First-phase DMA triggers in critical-path order (SP issues
            # them serially at ~600ns each): the very first matmul chain
            # needs only the m=0 y-unit weights + the first half of qt, so
            # those land first; activations are split into half tiles so
            # the chain starts after 256KB instead of 1MB.
            pending = {}

            def wload(which, w_dram, m):
                wt = wts.tile([128, 2, KC, 128], xdt, tag="wt")
                nc.sync.dma_start(wt[:], w_dram[m])
                pending[(which, m)] = (wt[:, 0], wt[:, 1])

            wty0 = wts.tile([128, KC, 128], xdt, tag="wty0")
            nc.sync.dma_start(wty0[:], wq_d[0, :, 0])
            qtA = big.tile([128, KC // 2, BR], xdt, tag="qtA")
            nc.sync.dma_start(qtA[:], qt_d[:, : KC // 2, :])
            qtB = big.tile([128, KC // 2, BR], xdt, tag="qtB")
            nc.sync.dma_start(qtB[:], qt_d[:, KC // 2 :, :])
            wtg0 = wts.tile([128, KC, 128], xdt, tag="wtg0")
            nc.sync.dma_start(wtg0[:], wq_d[0, :, 1])
            pending[("q", 0)] = (wty0, wtg0)
            wload("q", wq_d, 1)

            consts = sm.tile([128, 5 * MC], F32, tag="consts")
            nc.sync.dma_start(consts[:], consts_d[:])
            bsb = {
                n: consts[:, i * MC : (i + 1) * MC]
                for i, n in enumerate(("bqy", "bqg", "bhy", "bhg"))
            }
            watt = consts[:, 4 * MC : 5 * MC]
            consts2 = sm.tile([128, 256], BF16, tag="consts2")
            nc.sync.dma_start(consts2[:], consts2_d[:])
            eye = consts2[:, :128]
            maskTb = consts2[:, 128:]

            warm = sm.tile([128, 1], F32, tag="warm")
            nc.vector.memset(warm[:], 0.5)
            wout = sm.tile([128, 1], F32, tag="wout")
            ones = sm.tile([128, 1], BF16, tag="ones")
            nc.vector.memset(ones[:], 1.0)

            he = big.tile([128, MC, BR], EDT, tag="he")
            he2 = big.tile([128, MC, BR], EDT, tag="he2")
            qew = big.tile([128, MC, BR], EDT, tag="qew")
            qe2 = big.tile([128, MC, BR], EDT, tag="qe2")

            with (
                tc.tile_pool(name="pse", bufs=2, space="PSUM") as pse,
                tc.tile_pool(name="psnd", bufs=1, space="PSUM") as psnd,
            ):
                numT_ps = [
                    psnd.tile([128, 128], F32, name=f"num{g}", tag=f"num{g}")
                    for g in range(2)
                ]
                denT_ps = [
                    psnd.tile([128, 128], F32, name=f"den{g}", tag=f"den{g}")
                    for g in range(2)
                ]

                def embed_mm(ps, wt, xtA, xtB):
                    HK = KC // 2
                    if fp8:
                        for j in range(KC // 2):
                            xt = xtA if 2 * j < HK else xtB
                            o = 0 if 2 * j < HK else HK
                            nc.tensor.matmul(
                                ps[:],
                                wt[:, 2 * j : 2 * j + 2, :],
                                xt[:, 2 * j - o : 2 * j + 2 - o, :],
                                start=(j == 0),
                                stop=(j == KC // 2 - 1),
                                perf_mode=mybir.MatmulPerfMode.DoubleRow,
                            )
                    else:
                        for k in range(KC):
                            xt = xtA if k < HK else xtB
                            nc.tensor.matmul(
                                ps[:], wt[:, k, :], xt[:, k % HK, :],
                                start=(k == 0), stop=(k == KC - 1),
                            )

                def gated(xtA, xtB, which, by, bg, m, halves=(slice(0, BR),)):
                    """Consumes the prefetched weight tile for (which, m).
                    `halves` splits the post-GEMM elementwise work so a
                    row-group's outputs are ready earlier (tail chunk)."""
                    wty, wtg = pending.pop((which, m))
                    psy = pse.tile([128, BR], F32, tag="psy")
                    embed_mm(psy, wty, xtA, xtB)
                    psg = pse.tile([128, BR], F32, tag="psg")
                    embed_mm(psg, wtg, xtA, xtB)
                    ty = tmp.tile([128, BR], TDT, tag="ty")
                    tg = tmp.tile([128, BR], TDT, tag="tg")
                    for h in halves:
                        nc.scalar.activation(
                            ty[:, h], psy[:, h], ACT.Tanh,
                            bias=(0.0 if zero_bias else by[:, m : m + 1]),
                            scale=s,
                        )
                        if zero_bias:
                            # lrelu(s*z) = 0.01*s*z + 0.99*relu(s*z)
                            r = tmp.tile([128, BR], F32, tag="r")
                            nc.scalar.activation(
                                r[:, h], psg[:, h], ACT.Relu, scale=0.99 * s
                            )
                            nc.vector.scalar_tensor_tensor(
                                tg[:, h], psg[:, h], 0.01 * s, r[:, h],
                                op0=ALU.mult, op1=ALU.add,
                            )
                        else:
                            # lrelu(s*z + b) = max(a, 0.01a), a = s*z + b
                            a = tmp.tile([128, BR], F32, tag="r")
                            nc.scalar.activation(
                                a[:, h], psg[:, h], ACT.Identity,
                                bias=bg[:, m : m + 1], scale=s,
                            )
                            t1 = tmp.tile([128, BR], F32, tag="t1")
                            nc.gpsimd.tensor_scalar_mul(t1[:, h], a[:, h], 0.01)
                            nc.vector.tensor_max(tg[:, h], a[:, h], t1[:, h])
                    return ty, tg

                # ques embeddings
                for m in range(MC):
                    ty, tg = gated(qtA, qtB, "q", bsb["bqy"], bsb["bqg"], m)
                    if m + 2 < MC:
                        wload("q", wq_d, m + 2)
                    elif m + 2 < MC + 2:
                        wload("h", wh_d, m + 2 - MC)
                    nc.vector.scalar_tensor_tensor(
                        qew[:, m, :], ty[:], watt[:, m : m + 1], tg[:],
                        op0=ALU.mult, op1=ALU.mult,
                    )
                    qe = tmp.tile([128, BR], TDT, tag="qe")
                    nc.gpsimd.tensor_mul(qe[:], ty[:], tg[:])
                    nc.gpsimd.tensor_mul(qe2[:, m, :], qe[:], qe[:])
                    if m == 0:
                        # hist-transposed activations: stream during ques phase
                        htA = big.tile([128, KC // 2, BR], xdt, tag="htA")
                        nc.sync.dma_start(htA[:], ht_d[:, : KC // 2, :])
                    if m == 2:
                        htB = big.tile([128, KC // 2, BR], xdt, tag="htB")
                        nc.sync.dma_start(htB[:], ht_d[:, KC // 2 :, :])

                # hist embeddings + transposed num/den accumulation, lagged
                # so the tensor engine never waits on the vector engine's
                # he/he2 production. In fp8 mode the embeddings are e4m3 and
                # the num/den matmuls run DoubleRow over m-pairs.
                def numden(m, groups=(0, 1)):
                    last = m == MC - 1
                    for g in groups:
                        sl = slice(128 * g, 128 * (g + 1))
                        nc.tensor.matmul(
                            numT_ps[g][:], he[:, m, sl], qew[:, m, sl],
                            start=(m == 0), stop=False,
                        )
                        if last:
                            # fold the additive causal mask into the num
                            # PSUM (eye.T @ maskT = maskT) so the score
                            # chain skips a vector add
                            nc.tensor.matmul(
                                numT_ps[g][:], eye, maskTb,
                                start=False, stop=True,
                            )
                        nc.tensor.matmul(
                            denT_ps[g][:], he2[:, m, sl], qe2[:, m, sl],
                            start=(m == 0), stop=last,
                        )

                for m in range(MC):
                    halves = (
                        (slice(0, 128), slice(128, 256))
                        if m == MC - 1 else (slice(0, BR),)
                    )
                    ty, tg = gated(
                        htA, htB, "h", bsb["bhy"], bsb["bhg"], m, halves
                    )
                    if m + 2 < MC:
                        wload("h", wh_d, m + 2)
                    if m == MC - 1:
                        # previous chunk's lagged num/den first
                        numden(m - 1)
                        # tail-critical chunk: process per row-group so g0's
                        # num/den and score chain overlap g1's elementwise
                        for g in range(2):
                            sl = slice(128 * g, 128 * (g + 1))
                            nc.vector.tensor_mul(
                                he[:, m, sl], ty[:, sl], tg[:, sl]
                            )
                            nc.vector.tensor_mul(
                                he2[:, m, sl], he[:, m, sl], he[:, m, sl]
                            )
                            numden(MC - 1, groups=(g,))
                        continue
                    nc.vector.tensor_mul(he[:, m, :], ty[:], tg[:])
                    nc.vector.tensor_mul(he2[:, m, :], he[:, m, :], he[:, m, :])
                    if m == 5:
                        # feat inputs: late enough not to starve the weight
                        # stream, early enough for the attention tail
                        hn = big.tile([128, 2, IN], BF16, tag="hn")
                        nc.sync.dma_start(hn[:], hn_d[:])
                    if m == 6:
                        # preload the Sqrt activation table off the critical
                        # path (ACT_TABLE_LOAD costs ~1.3us; the engine seems
                        # to keep the Tanh/Relu table plus one swap slot, so
                        # Exp is preloaded separately after the sqrt uses)
                        nc.scalar.activation(wout[:], warm[:], ACT.Sqrt)
                    if m > 0:
                        numden(m - 1)

                # masked scores while num/den PSUM is still allocated
                sc = []
                for g in range(2):
                    # num carries the ASCALE factor from qew; fold the
                    # matching 1/ASCALE into sd = sqrt(ASCALE^2 * den)
                    sd = tmp.tile([128, 128], F32, tag="sd")
                    nc.scalar.activation(
                        sd[:], denT_ps[g][:], ACT.Sqrt,
                        scale=(ASCALE * ASCALE if fp8 else 1.0),
                    )
                    rdT = tmp.tile([128, 128], F32, tag="rdT")
                    nc.vector.reciprocal(rdT[:], sd[:])
                    sT = sm.tile([128, 128], F32, name=f"sT{g}", tag=f"sT{g}")
                    nc.vector.tensor_mul(sT[:], numT_ps[g][:], rdT[:])
                    sc.append(sT)
                # preload the Exp table while vector finishes the scores
                nc.scalar.activation(wout[:], warm[:], ACT.Exp)

            # attention tail: attU^T = exp(sT); feat is shipped UNNORMALIZED
            # together with the attention row sums (ones-column matmul) and
            # the host divides, so the output copies have no dependency on
            # the row-sum reduction.
            with (
                tc.tile_pool(name="psa", bufs=2, space="PSUM") as psa,
                tc.tile_pool(name="psf", bufs=4, space="PSUM") as psf,
            ):
                # dummy matmuls with no data deps: keep the PE pstate hot
                # while scalar/vector run the score chain, so the feat
                # matmuls don't pay the downclocked rate
                hot = psa.tile([128, 128], F32, tag="hot")
                for _ in range(24):
                    nc.tensor.matmul(hot[:], eye, maskTb, start=True, stop=True)
                attTs = []
                for g in range(2):
                    attT = sm.tile([128, 128], BF16, name=f"attT{g}", tag=f"attT{g}")
                    nc.scalar.activation(attT[:], sc[g][:], ACT.Exp)
                    attTs.append(attT)
                for g in range(2):
                    attT = attTs[g]
                    fsb = tmp.tile([128, IN + 16], BF16, tag="fsb")
                    rs_ps = psa.tile([128, 1], F32, tag="rs")
                    nc.tensor.matmul(rs_ps[:], attT[:], ones[:])
                    nc.scalar.copy(fsb[:, IN : IN + 1], rs_ps[:])
                    for c in range(4):
                        cs = slice(512 * c, 512 * (c + 1))
                        fps = psf.tile([128, 512], F32, tag="fps")
                        nc.tensor.matmul(
                            fps[:], attT[:], hn[:, g, cs],
                            start=True, stop=True,
                        )
                        if c % 2 == 0:
                            nc.scalar.copy(fsb[:, cs], fps[:])
                        else:
                            nc.vector.tensor_copy(fsb[:, cs], fps[:])
                        if c == 1:
                            nc.sync.dma_start(
                                feat_d[g, :, :1024], fsb[:, :1024]
                            )
                    nc.sync.dma_start(feat_d[g, :, 1024:], fsb[:, 1024:])

    _split_multi_waits(nc)
    return nc


# ---------------------------------------------------------------------------
# Host side
# ---------------------------------------------------------------------------

_PROG_CACHE = {}


def _get_prog(mode, zero_bias):
    key = (mode, zero_bias)
    if key not in _PROG_CACHE:
        _PROG_CACHE[key] = build_program(mode, zero_bias)
    return _PROG_CACHE[key]


def _prep_shared(W_hy, b_hy, W_hg, b_hg, W_qy, b_qy, W_qg, b_qg, W_att, mode):
    fp8 = mode == "fp8"
    xnp = ml_dtypes.float8_e4m3 if fp8 else ml_dtypes.bfloat16
    ws = WSCALE if fp8 else 1.0

    def reblock(W):
        # [IN, H] -> [MC, 128, KC, 128]; Wr[m, p, k, h] = W[128k+p, 128m+h]
        return np.ascontiguousarray(
            (W * ws).reshape(KC, 128, MC, 128).transpose(2, 1, 0, 3)
        ).astype(xnp)

    def bvec(b):
        return np.ascontiguousarray(b.reshape(MC, 128).T).astype(np.float32)

    # transposed causal mask: maskT[h, q] = 0 if h <= q (same example), -inf-ish
    # otherwise; off-diagonal 32x32 blocks fully masked.
    maskT = np.full((128, 128), NEG, np.float32)
    m32T = np.where(
        np.arange(32)[:, None] <= np.arange(32)[None, :], 0.0, NEG
    ).astype(np.float32)
    for e in range(4):
        maskT[32 * e : 32 * (e + 1), 32 * e : 32 * (e + 1)] = m32T
    consts2 = np.concatenate(
        [np.eye(128, dtype=np.float32), maskT], axis=1
    ).astype(ml_dtypes.bfloat16)

    # [MC, 128, 2, KC, 128]
    wh = np.ascontiguousarray(np.stack([reblock(W_hy), reblock(W_hg)], axis=2))
    wq = np.ascontiguousarray(np.stack([reblock(W_qy), reblock(W_qg)], axis=2))
    ascale = ASCALE if fp8 else 1.0
    consts = np.concatenate(
        [bvec(b_qy), bvec(b_qg), bvec(b_hy), bvec(b_hg),
         bvec(W_att * ascale)],
        axis=1,
    )
    shared = {
        "wh": wh,
        "wq": wq,
        "consts": np.ascontiguousarray(consts),
        "consts2": np.ascontiguousarray(consts2),
    }
    return shared, xnp


def kernel(
    hist, ques, W_hy, b_hy, W_hg, b_hg, W_qy, b_qy, W_qg, b_qg, W_att, b_att,
    mode="fp8", trace=False,
):
    from concourse.bass_utils import run_bass_kernel_spmd

    hist = np.asarray(hist, np.float32)
    ques = np.asarray(ques, np.float32)
    zero_bias = all(
        not np.any(np.asarray(b)) for b in (b_hy, b_hg, b_qy, b_qg)
    )
    nc = _get_prog(mode, zero_bias)
    shared, xnp = _prep_shared(
        np.asarray(W_hy, np.float32), np.asarray(b_hy, np.float32),
        np.asarray(W_hg, np.float32), np.asarray(b_hg, np.float32),
        np.asarray(W_qy, np.float32), np.asarray(b_qy, np.float32),
        np.asarray(W_qg, np.float32), np.asarray(b_qg, np.float32),
        np.asarray(W_att, np.float32), mode,
    )

    def pmaj(x2d):
        # [BR, IN] -> [128, KC, BR]: out[p, k, b] = x2d[b, 128k+p]
        return np.ascontiguousarray(
            x2d.T.reshape(KC, 128, BR).transpose(1, 0, 2)
        ).astype(xnp)

    in_maps = []
    for c in range(NCORES):
        hs = hist[c * BL : (c + 1) * BL].reshape(BR, IN)
        qs = ques[c * BL : (c + 1) * BL].reshape(BR, IN)
        im = dict(shared)
        im["qt"] = pmaj(qs)
        im["ht"] = pmaj(hs)
        im["hn"] = np.ascontiguousarray(
            hs.reshape(2, 128, IN).transpose(1, 0, 2)
        ).astype(ml_dtypes.bfloat16)
        in_maps.append(im)

    res = run_bass_kernel_spmd(
        nc, in_maps, core_ids=list(range(NCORES)), trace=trace
    )
    def unpack(r):
        o = r["feat"].astype(np.float32).reshape(BR, IN + 16)
        return (o[:, :IN] / o[:, IN : IN + 1]).reshape(BL, R, IN)

    feat = np.concatenate([unpack(r) for r in res.results], axis=0)
    if trace:
        return feat, res
    return feat
